# revision 1
# baseline (speedup 1.0000x reference)
"""GroupEmbedding kernel for Trainium2 (8 NeuronCores, Bass/Tile).

  beh_emb      = item_table[behavior_item_ids] * behavior_counts[:,None]
  per_user_beh = segment_sum(beh_emb, behavior_user_ids, n_users)
  ue           = user_table[user_ids] * (user_ids != 0)
  per_user     = per_user_beh * ue
  out          = segment_sum(per_user, user_group_ids, num_groups)

Sharding: data parallel on the ragged behavior axis; each user's behaviors
live on one core.  Users are bin-packed onto (core, window, slot) = 8 x 64 x
128 so every window's per-item-table-quarter behavior counts are balanced
(near-zero gather padding vs offset-aligned splits).

Per (window-pair, quarter) the behaviors stream through fp16 dma_gathers
whose 512B descriptors each fetch TWO consecutive item-table rows (overlapping
access pattern, elem_step = one row): behaviors of the same window with
table-adjacent items share one descriptor, the partner riding the upper row
half.  The stream is laid out [w0 pairs][w1 pairs][w0 singles][w1 singles]
with w1 regions starting unaligned right after w0's cross-core max, and a
zero row is appended to the table so the trailing row-pair read stays in
bounds.  Per 128-slot tile a fused tensor_scalar (is_equal x mult, fp16,
DVE 4x mode; pair-region upper halves on the Pool engine) builds the
count-scaled one-hot routing matrix -- window membership is encoded +128 in
loc so mixed tiles disambiguate -- and fp16 PE matmuls accumulate per-user
sums in fp32 PSUM.  Descriptor counts are exact (no tile rounding); a
high-water mark per gather buffer guarantees no slot is read uninitialized.

Window epilogue: indirect-DMA gathers the 128 user embeddings, multiplies
them in on DVE, and stages fp16 per-user slabs that flush to DRAM in
8-window 2KB-descriptor batches.  Host applies the final group segment-sum
(the cross-shard psum) with one np.add.at over the 65536 per-user rows.
"""

import sys

sys.path.insert(0, "/opt/trn_rl_repo")

import numpy as np

P = 128
EMB = 128
N_CORES = 8
N_USERS = 65536
WPC = 64          # windows per core
NPAIR = WPC // 2  # gather pair-slots per core
CH = 25600        # item-table quarter size; local indices fit int16
NQ = 4


def _build_program(P0, P1, R0, R1, item_rows, user_rows):
    """Rp/R0/R1: [NPAIR, NQ] cross-core max slot counts per (pair, quarter)
    stream region: [item-adjacent behavior pairs][w0 singles][w1 singles].
    Every descriptor fetches TWO consecutive fp16 table rows (512B, full
    rate); a pair slot's second behavior rides the same descriptor (hi half).
    Window membership is encoded in loc (+128 for w1), so any tile can mix
    windows; pair-region tiles also issue hi-half sel-matmuls."""
    from concourse import bacc, mybir
    import concourse.bass as bass
    import concourse.tile as tile
    import dataclasses

    dt = mybir.dt
    Alu = mybir.AluOpType
    P0, P1 = np.asarray(P0), np.asarray(P1)
    R0, R1 = np.asarray(R0), np.asarray(R1)
    Rp = P0 + P1
    L_sq = Rp + R0 + R1                           # [NPAIR, NQ] stream slots
    T_sq = -(-L_sq // P)                          # [NPAIR, NQ] tiles
    NT = int(T_sq.sum())
    npt_sq = -(-Rp // P)                          # tiles carrying hi entries
    NTp = int(npt_sq.sum())
    hi_off = np.zeros((NPAIR, NQ), np.int64)
    acc_h = 0
    for s in range(NPAIR):
        for q in range(NQ):
            hi_off[s, q] = acc_h
            acc_h += int(npt_sq[s, q])

    nc = bacc.Bacc(None, target_bir_lowering=False)
    item_t = nc.dram_tensor("item16", [item_rows, EMB], dt.float16, kind="ExternalInput")
    user_t = nc.dram_tensor("user_table", [user_rows, EMB], dt.float32, kind="ExternalInput")
    beh_idx = nc.dram_tensor("beh_idx", [P, NT * 8], dt.int16, kind="ExternalInput")
    beh_cnt = nc.dram_tensor("beh_cnt", [P, NT], dt.float32, kind="ExternalInput")
    beh_loc = nc.dram_tensor("beh_loc", [P, NT], dt.float32, kind="ExternalInput")
    hi_cnt = nc.dram_tensor("hi_cnt", [P, max(NTp, 1)], dt.float32, kind="ExternalInput")
    hi_loc = nc.dram_tensor("hi_loc", [P, max(NTp, 1)], dt.float32, kind="ExternalInput")
    win_uid = nc.dram_tensor("win_uid", [P, WPC], dt.int32, kind="ExternalInput")
    iota_in = nc.dram_tensor("iota16", [P, P], dt.float16, kind="ExternalInput")
    gout = nc.dram_tensor("gout", [P, WPC * EMB], dt.float16, kind="ExternalOutput")

    # column offset of each (pair, quarter) run in the tile stream
    off_sq = np.zeros((NPAIR, NQ), np.int64)
    acc = 0
    for s in range(NPAIR):
        for q in range(NQ):
            off_sq[s, q] = acc
            acc += int(T_sq[s, q])
    assert acc == NT

    with tile.TileContext(nc) as tc:
        with (
            tc.tile_pool(name="meta", bufs=1) as meta_tp,
            tc.tile_pool(name="gbuf", bufs=3) as gbuf_tp,
            tc.tile_pool(name="sel", bufs=16) as sel_tp,
            tc.tile_pool(name="epi", bufs=2) as epi_tp,
            tc.tile_pool(name="upsum", bufs=4, space="PSUM") as upsum_tp,
        ):
            idx_s = meta_tp.tile([P, NT * 8], dt.int16)
            nc.sync.dma_start(idx_s[:], beh_idx[:])
            idx_tiles = [(idx_s, 0)] * NPAIR
            cnt_s = meta_tp.tile([P, NT], dt.float32)
            nc.sync.dma_start(cnt_s[:], beh_cnt[:])
            loc_s = meta_tp.tile([P, NT], dt.float32)
            nc.sync.dma_start(loc_s[:], beh_loc[:])
            # loc for the second window of each pair: stored +128 in the
            # plane, so loc_s never matches iota (0..127) for w1 rows and
            # loc_b = loc_s - 128 never matches for w0 rows
            loc_b = meta_tp.tile([P, NT], dt.float32)
            nc.vector.tensor_scalar(out=loc_b[:], in0=loc_s[:], scalar1=-128.0,
                                    scalar2=None, op0=Alu.add)
            hcnt_s = meta_tp.tile([P, max(NTp, 1)], dt.float32)
            nc.sync.dma_start(hcnt_s[:], hi_cnt[:])
            hloc_s = meta_tp.tile([P, max(NTp, 1)], dt.float32)
            nc.sync.dma_start(hloc_s[:], hi_loc[:])
            hloc_b = meta_tp.tile([P, max(NTp, 1)], dt.float32)
            nc.vector.tensor_scalar(out=hloc_b[:], in0=hloc_s[:], scalar1=-128.0,
                                    scalar2=None, op0=Alu.add)
            uid_s = meta_tp.tile([P, WPC], dt.int32)
            nc.sync.dma_start(uid_s[:], win_uid[:])
            iota_s = meta_tp.tile([P, P], dt.float16)
            nc.sync.dma_start(iota_s[:], iota_in[:])

            # per-(tag, buffer) high-water mark: a gather writes all T*128
            # slots whenever its tile extends past everything that buffer has
            # held, so no slot is ever read uninitialized
            hiwater = {}
            GRP = 8
            stages = []
            for s in range(NPAIR):
                upsum_a = upsum_tp.tile([P, EMB], dt.float32, tag="upsum_a")
                upsum_b = upsum_tp.tile([P, EMB], dt.float32, tag="upsum_b")
                upsums = [upsum_a, upsum_b]
                # per-tile sel-matmul work list: (q, t, row_half, win_half)
                work = []
                for q in range(NQ):
                    p0, rp = int(P0[s, q]), int(Rp[s, q])
                    r0 = int(R0[s, q])
                    L = int(L_sq[s, q])
                    for t in range(int(T_sq[s, q])):
                        a, b2 = t * P, (t + 1) * P
                        if a < p0:                       # w0 pairs
                            work.append((q, t, 1, 0))
                        if b2 > p0 and a < rp:           # w1 pairs
                            work.append((q, t, 1, 1))
                        if a < p0 or (b2 > rp and a < rp + r0):
                            work.append((q, t, 0, 0))    # w0 lo (pairs+singles)
                        if (b2 > p0 and a < rp) or (b2 > rp + r0 and a < L):
                            work.append((q, t, 0, 1))    # w1 lo (pairs+singles)
                n_mm = [sum(1 for ww in work if ww[3] == h) for h in range(2)]
                done = [0, 0]
                # interleave per-quarter: gather q, then its tiles' matmuls
                for q in range(NQ):
                    T_q = int(T_sq[s, q])
                    if T_q == 0:
                        continue
                    t0 = int(off_sq[s, q])
                    h0 = int(hi_off[s, q])
                    it, ib = idx_tiles[s]
                    hw_key = (q, s % 3)
                    ni = int(L_sq[s, q])
                    if T_q * P > hiwater.get(hw_key, 0):
                        ni = T_q * P              # full write on new high-water
                        hiwater[hw_key] = T_q * P
                    ncol = -(-ni // 16)
                    gb = gbuf_tp.tile([P, T_q, 2 * EMB], dt.float16, tag=f"gb{q}")
                    in_full = item_t[q * CH : min((q + 1) * CH, item_rows - 1) + 1, :]
                    in_ov = dataclasses.replace(
                        in_full, ap=[[EMB, in_full.ap[0][1] - 1], [1, 2 * EMB]])
                    nc.gpsimd.dma_gather(
                        gb[:],
                        in_ov,
                        it[:, t0 * 8 - ib : t0 * 8 - ib + ncol],
                        ni,
                        ni,
                        2 * EMB,
                        elem_step=EMB,
                        single_packet=False,
                    )
                    for (qq, t, rh, wh) in work:
                        if qq != q:
                            continue
                        if rh == 0:
                            locp = [loc_s, loc_b][wh][:, t0 + t : t0 + t + 1]
                            cntp = cnt_s[:, t0 + t : t0 + t + 1]
                            eng = nc.vector
                        else:
                            locp = [hloc_s, hloc_b][wh][:, h0 + t : h0 + t + 1]
                            cntp = hcnt_s[:, h0 + t : h0 + t + 1]
                            # hi-half sels split across engines: DVE's issue
                            # rate (~139ns/op) is the serial chain and Pool
                            # has limited slack between gather desc gens
                            eng = nc.gpsimd
                        sel = sel_tp.tile([P, P], dt.float16, tag="sel")
                        eng.tensor_scalar(
                            out=sel[:], in0=iota_s[:],
                            scalar1=locp, scalar2=cntp,
                            op0=Alu.is_equal, op1=Alu.mult,
                        )
                        nc.tensor.matmul(
                            out=upsums[wh][:],
                            lhsT=sel[:],
                            rhs=gb[:, t, rh * EMB : (rh + 1) * EMB],
                            start=(done[wh] == 0),
                            stop=(done[wh] == n_mm[wh] - 1),
                        )
                        done[wh] += 1
                for half in range(2):
                    w = 2 * s + half
                    # epilogue: user embeddings, per-user mult; pu slabs land
                    # in a staging tile flushed GRP windows at a time so the
                    # gout descriptors are 2KB (full-rate) instead of 256B
                    ue = epi_tp.tile([P, EMB], dt.float32, tag="ue")
                    nc.gpsimd.indirect_dma_start(
                        out=ue[:],
                        out_offset=None,
                        in_=user_t[:],
                        in_offset=bass.IndirectOffsetOnAxis(ap=uid_s[:, w : w + 1], axis=0),
                    )
                    g, wl = w // GRP, w % GRP
                    if wl == 0:
                        stage_t = epi_tp.tile([P, GRP * EMB], dt.float16, tag="stage")
                        stages.append(stage_t)
                    nc.vector.tensor_tensor(out=stages[g][:, wl * EMB : (wl + 1) * EMB],
                                            in0=upsums[half][:], in1=ue[:], op=Alu.mult)
                    if wl == GRP - 1:
                        nc.sync.dma_start(gout[:, g * GRP * EMB : (g + 1) * GRP * EMB],
                                          stages[g][:])
    nc.finalize()
    return nc


def _pack_users(behavior_item_ids, behavior_user_ids):
    """Assign users -> (core, window, slot) balancing per-quarter behavior
    counts into 128-aligned tile budgets. Returns assignment + per-core
    per-window per-quarter behavior counts."""
    q = (behavior_item_ids // CH).astype(np.int64)
    uq = np.bincount(behavior_user_ids.astype(np.int64) * NQ + q,
                     minlength=N_USERS * NQ).reshape(N_USERS, NQ)
    tot = uq.sum(1)

    # users -> cores: greedy LPT on per-quarter vectors so every core's
    # quarter totals land within a few users of the mean
    order = np.argsort(-tot, kind="stable")
    core_of = np.empty(N_USERS, np.int64)
    UPC = N_USERS // N_CORES
    target_q = uq.sum(0) / N_CORES
    cq = np.zeros((N_CORES, NQ), np.float64)
    cn = np.zeros(N_CORES, np.int64)
    for u in order:
        score = ((cq + uq[u]) / target_q).max(1)
        score[cn >= UPC] = np.inf
        c = int(np.argmin(score))
        core_of[u] = c
        cq[c] += uq[u]
        cn[c] += 1

    # common per-(window, quarter) tile budget across cores (+1 slack tile/q)
    Qcq = np.zeros((N_CORES, NQ), np.int64)
    for c in range(N_CORES):
        Qcq[c] = uq[core_of == c].sum(0)
    Kq = -(-Qcq.max(0) // P) + 3
    b_wq = np.tile(Kq // WPC, (WPC, 1))
    # stagger the +1-tile quarters across windows to equalize window totals
    for qq in range(NQ):
        extra = int(Kq[qq] % WPC)
        if extra:
            order_w = np.argsort(b_wq.sum(1), kind="stable")
            b_wq[order_w[:extra], qq] += 1
    cap = b_wq * P                                    # behavior capacity

    win_of = np.empty(N_USERS, np.int64)
    slot_of = np.empty(N_USERS, np.int64)
    load_all = np.zeros((N_CORES, WPC, NQ), np.int64)

    for c in range(N_CORES):
        us = np.where(core_of == c)[0]
        us = us[np.argsort(-tot[us], kind="stable")]
        load = np.zeros((WPC, NQ), np.int64)
        nuser = np.zeros(WPC, np.int64)
        for u in us:
            v = uq[u]
            over = np.maximum(load + v - cap, 0).sum(1)
            feas = (nuser < P) & (over == 0)
            if feas.any():
                # least-relatively-full feasible window: windows fill
                # proportionally to their budgets, so the small tail fits
                rel = ((load + v) / cap).max(1)
                w = int(np.argmin(np.where(feas, rel, np.inf)))
            else:
                over[nuser >= P] = 1 << 62
                w = int(np.argmin(over))
            slot_of[u] = nuser[w]
            nuser[w] += 1
            win_of[u] = w
            load[w] += v
        assert (nuser == P).all()
        load_all[c] = load

    # cross-core tile profile: budgets are slot-aligned across cores, so the
    # max only exceeds b_wq where a core's packing overflowed
    k_wq = (-(-load_all // P)).max(0)                 # [WPC, NQ]
    return core_of, win_of, slot_of, k_wq, load_all


def _prepare(user_ids, user_group_ids, behavior_item_ids, behavior_counts,
             behavior_user_ids):
    core_of, win_of, slot_of, k_wq, load_all = _pack_users(behavior_item_ids,
                                                           behavior_user_ids)
    q = (behavior_item_ids // CH).astype(np.int64)
    bu = behavior_user_ids.astype(np.int64)
    bc = core_of[bu]
    bw = win_of[bu]
    bs = slot_of[bu]

    # order behaviors by (core, pair, quarter), item-sorted within each run
    NR = N_CORES * NPAIR * NQ
    key = (bc * NPAIR + bw // 2) * NQ + q
    order = np.lexsort((behavior_item_ids, key))
    key_s = key[order]
    item_s = behavior_item_ids[order].astype(np.int64)
    runs = np.bincount(key_s, minlength=NR)
    starts = np.concatenate([[0], np.cumsum(runs)[:-1]])
    run_id = key_s

    # greedy non-overlapping pairing of item-adjacent consecutive behaviors
    n = len(order)
    elig = np.zeros(n, bool)
    whm = (bw[order] % 2).astype(np.int64)
    elig[:-1] = ((item_s[1:] == item_s[:-1] + 1) & (key_s[1:] == key_s[:-1])
                 & (whm[1:] == whm[:-1]))
    prev = np.concatenate([[False], elig[:-1]])
    idxs = np.arange(n)
    cs = np.maximum.accumulate(np.where(elig & ~prev, idxs, -1))
    isA = elig & ((idxs - cs) % 2 == 0)
    isB = np.concatenate([[False], isA[:-1]])
    sing = ~(isA | isB)
    wh = (bw[order] % 2).astype(np.int64)

    def seg_rank(mask):
        c = np.cumsum(mask)
        base = (c - mask)[starts[run_id]]          # cumsum before run start
        return c - mask - base                     # 0-based rank within run

    # pairs keep the window of their A behavior; both members route by loc
    whA = np.where(isB, np.concatenate([[0], wh[:-1]]), wh)
    pr0 = seg_rank(isA & (whA == 0))
    pr1 = seg_rank(isA & (whA == 1))
    r0 = seg_rank(sing & (wh == 0))
    r1 = seg_rank(sing & (wh == 1))

    np0 = np.bincount(run_id[isA & (whA == 0)], minlength=NR).reshape(N_CORES, NPAIR, NQ)
    np1 = np.bincount(run_id[isA & (whA == 1)], minlength=NR).reshape(N_CORES, NPAIR, NQ)
    ns0 = np.bincount(run_id[sing & (wh == 0)], minlength=NR).reshape(N_CORES, NPAIR, NQ)
    ns1 = np.bincount(run_id[sing & (wh == 1)], minlength=NR).reshape(N_CORES, NPAIR, NQ)
    P0, P1, R0, R1 = np0.max(0), np1.max(0), ns0.max(0), ns1.max(0)
    Rp = P0 + P1
    L_sq = Rp + R0 + R1
    T_sq = -(-L_sq // P)
    NT = int(T_sq.sum())
    npt_sq = -(-Rp // P)
    NTp = int(npt_sq.sum())
    off_sq = np.zeros((NPAIR, NQ), np.int64)
    hi_off = np.zeros((NPAIR, NQ), np.int64)
    acc = acc_h = 0
    for s in range(NPAIR):
        for qq in range(NQ):
            off_sq[s, qq] = acc
            acc += int(T_sq[s, qq])
            hi_off[s, qq] = acc_h
            acc_h += int(npt_sq[s, qq])

    # stream slot of each behavior: A and B share the pair slot; singles go
    # after the pair region (w0 then w1)
    sq_flat = (bw[order] // 2) * NQ + q[order]
    slot = np.where(isA & (whA == 0), pr0, 0)
    slot = np.where(isA & (whA == 1), P0.reshape(-1)[sq_flat] + pr1, slot)
    slotA = np.concatenate([[0], slot[:-1]])
    slot = np.where(isB, slotA, slot)
    slot = np.where(sing & (wh == 0), Rp.reshape(-1)[sq_flat] + r0, slot)
    slot = np.where(sing & (wh == 1),
                    (Rp + R0).reshape(-1)[sq_flat] + r1, slot)
    assert (slot < L_sq.reshape(-1)[sq_flat]).all()
    g_pos = off_sq.reshape(-1)[sq_flat] * P + slot
    t_glob = g_pos // P
    p_in = g_pos % P
    core_s = bc[order]

    # lo planes carry A + singles; hi planes carry B (pair-region tiles only)
    lo = ~isB
    locv = (bs[order] + wh * P).astype(np.float32)
    cnt_plane = np.zeros((N_CORES, P, NT), np.float32)
    loc_plane = np.zeros((N_CORES, P, NT), np.float32)
    flat = (core_s * (P * NT) + p_in * NT + t_glob)[lo]
    cnt_plane.reshape(-1)[flat] = behavior_counts[order][lo]
    loc_plane.reshape(-1)[flat] = locv[lo]

    NTp1 = max(NTp, 1)
    hcnt_plane = np.zeros((N_CORES, P, NTp1), np.float32)
    hloc_plane = np.zeros((N_CORES, P, NTp1), np.float32)
    t_hi = hi_off.reshape(-1)[sq_flat] + (slot // P)   # valid for B rows
    flat_h = (core_s * (P * NTp1) + p_in * NTp1 + t_hi)[isB]
    hcnt_plane.reshape(-1)[flat_h] = behavior_counts[order][isB]
    hloc_plane.reshape(-1)[flat_h] = locv[isB]

    # int16 gather-index plane (descriptor owners = lo rows only)
    idx_plane = np.zeros((N_CORES, 16, NT * 8), np.int16)
    col = t_glob * 8 + p_in // 16
    flat_i = (core_s * (16 * NT * 8) + (p_in % 16) * (NT * 8) + col)[lo]
    local_item = (behavior_item_ids[order].astype(np.int64) - q[order] * CH)
    idx_plane.reshape(-1)[flat_i] = local_item[lo].astype(np.int16)
    idx_plane = np.tile(idx_plane, (1, 8, 1))

    # per (core, window, slot): user position -> table id & group
    uid_plane = np.zeros((N_CORES, P, WPC), np.int32)
    grp_plane = np.zeros((N_CORES, P, WPC), np.int64)
    uu = np.arange(N_USERS)
    uid_plane[core_of, slot_of, win_of] = user_ids.astype(np.int32)
    grp_plane[core_of[uu], slot_of[uu], win_of[uu]] = user_group_ids.astype(np.int64)

    iota16 = np.broadcast_to(np.arange(P, dtype=np.float16), (P, P)).copy()
    return dict(idx_plane=idx_plane, cnt_plane=cnt_plane, loc_plane=loc_plane,
                hcnt_plane=hcnt_plane, hloc_plane=hloc_plane,
                uid_plane=uid_plane, grp_plane=grp_plane, iota16=iota16,
                P0=P0, P1=P1, R0=R0, R1=R1, NT=NT)


_CACHE = {}


def kernel(user_ids, user_group_ids, behavior_item_ids, behavior_counts,
           behavior_user_ids, user_table, item_table, num_groups):
    from concourse.bass_utils import run_bass_kernel_spmd

    user_ids = np.asarray(user_ids)
    user_group_ids = np.asarray(user_group_ids)
    behavior_item_ids = np.asarray(behavior_item_ids)
    behavior_counts = np.asarray(behavior_counts, dtype=np.float32)
    behavior_user_ids = np.asarray(behavior_user_ids)
    user_table = np.asarray(user_table, dtype=np.float32)
    item_table = np.asarray(item_table, dtype=np.float32)
    n_groups = int(num_groups)

    meta = _prepare(user_ids, user_group_ids, behavior_item_ids,
                    behavior_counts, behavior_user_ids)

    item16 = np.concatenate([item_table.astype(np.float16),
                             np.zeros((1, EMB), np.float16)])
    user_table_z = user_table.copy()
    user_table_z[0] = 0.0

    key = (tuple(meta["P0"].reshape(-1).tolist()),
           tuple(meta["P1"].reshape(-1).tolist()),
           tuple(meta["R0"].reshape(-1).tolist()),
           tuple(meta["R1"].reshape(-1).tolist()),
           item_table.shape[0], user_table.shape[0])
    if key not in _CACHE:
        _CACHE[key] = _build_program(meta["P0"], meta["P1"], meta["R0"], meta["R1"],
                                     item16.shape[0], user_table.shape[0])
    nc = _CACHE[key]

    in_maps = []
    for c in range(N_CORES):
        in_maps.append({
            "item16": item16,
            "user_table": user_table_z,
            "beh_idx": meta["idx_plane"][c],
            "beh_cnt": meta["cnt_plane"][c],
            "beh_loc": meta["loc_plane"][c],
            "hi_cnt": meta["hcnt_plane"][c],
            "hi_loc": meta["hloc_plane"][c],
            "win_uid": meta["uid_plane"][c],
            "iota16": meta["iota16"],
        })

    res = run_bass_kernel_spmd(nc, in_maps, core_ids=list(range(N_CORES)))
    out = np.zeros((n_groups, EMB), np.float32)
    for c in range(N_CORES):
        slab = res.results[c]["gout"].reshape(P, WPC, EMB).astype(np.float32)
        np.add.at(out, meta["grp_plane"][c].reshape(-1),
                  slab.reshape(P * WPC, EMB))
    return out

